# revision 34
# baseline (speedup 1.0000x reference)
"""Trainium2 Bass kernel for causal self-attention (T=2048, C=1024, NH=16).

Strategy (tensor-parallel over heads, 2 heads/core on 8 cores):
  - Host pre-transposes x, w_attn-slice, w_proj so all device matmuls have
    their contraction dim on SBUF partitions (no fp32 DMA transposes needed).
  - Per core: qkv projection in [ch, t] layout; attention computed as
    att_T = k @ q.T tiles ([t_k, t_q]) so softmax's denominator comes for free
    from an appended ones-column on v (no partition-axis reduction).
  - Software-pipelined emission: qkv round r+1's projection chains are
    injected into attention round r's pair loop, and each pair's att@v is
    delayed one pair, so the in-order PE queue always has filler behind
    exp/mask dependency stalls (PE >95% dense in the attention span).
  - Softmax without max-subtraction (inputs bounded, |att| < 8). Causality:
    block skipping, partial-N matmuls on diagonal blocks, and one shared
    128x128 triangular mask applied as a strided paired DVE multiply.
  - exp split across engines per head: head A (and all of round 0) uses
    ScalarE's Exp LUT; head B uses a custom 8-stage DVE op computing
    ((1+a/32)^2+1)^32 = 2^32*e^a*(1+O(a^3/3072)) -- constant factor and
    approximation are consistent within a (round, head), so softmax
    normalization cancels them (adds ~1e-3 rel err). Elementwise PSUM
    evacuation is likewise split: head A on ScalarE activations
    (bias/scale fused), head B on DVE.
  - The reference's bug-faithful reshape (NH,T,HD)->swap(1,2)->(T,C) makes the
    output row-parallel over heads: each core produces 256 full output rows,
    so there is no all-reduce at all; host concatenates (+ proj bias).
  - All bulk input DMA rides one HWDGE queue in exact need-order (wq slice,
    first half of x quarter 0, bias, rest of x, w_proj) because transfers
    serialize at the SDMA engines; ~3.4us of dummy matmuls warm the PE HAM
    clock gate during the wait. tile_position packing of the 64-deep q@k
    was tried and reverted: 64<->128 row-mode switches tax every matmul.
  - bf16 matmul operands (FWL-eligible 128-wide stationaries); fp32 PSUM.
"""
import math
import os

import numpy as np

import concourse.bass as bass
import concourse.bacc as bacc
import concourse.mybir as mybir
import concourse.tile as tile
from concourse import bass_utils
from concourse.masks import make_identity

T, C, NH, HD = 2048, 1024, 16, 64
P = 128
NCORES = 8
HPC = 2          # heads per core
F32 = mybir.dt.float32
MMDT = mybir.dt.bfloat16  # matmul input dtype
EXPF = mybir.ActivationFunctionType.Exp
COPYF = mybir.ActivationFunctionType.Copy
IDENT = mybir.ActivationFunctionType.Identity

PACK_QK = False   # concurrent 64-row tile_position q@k for the two heads

# head-B softmax approximation base: es = ((1 + a/32)^2 + 1)^32
BSH = 5          # squarings after the base
BDIV = float(2 ** BSH)   # 32


def _to_mm(a):
    import ml_dtypes
    return np.ascontiguousarray(np.asarray(a, dtype=np.float32).astype(ml_dtypes.bfloat16))


_EXP32_OP = None


def _register_exp32():
    """Register the custom DVE op ((1+x)^2+1)^32 (8 ALU stages) at runtime."""
    global _EXP32_OP
    if _EXP32_OP is not None:
        return _EXP32_OP
    import concourse.dve_ops as dve_ops
    from concourse.dve_spec import Spec, Src0, One, sq, lower as dve_lower
    from concourse.dve_table_gen import dve_ver_for
    from concourse.dve_uop import DveOpSpec

    name = "EXP32S_ANT"
    for op in dve_ops.OPS:
        if op.name == name:
            _EXP32_OP = op
            return op

    def _ref(in0, in1, s0, s1, imm2):
        t = np.asarray(in0, np.float32) + 1.0
        t = (t * t + 1.0).astype(np.float32)
        for _ in range(BSH):
            t = (t * t).astype(np.float32)
        return t

    body = sq(sq(sq(sq(sq(sq(Src0 + One) + One)))))
    spec = Spec(body=body, reference=_ref)
    row = dve_ops._CUSTOM_DVE_ROW_BASE + len(dve_ops.OPS)
    ver = dve_ver_for("TRN2")
    sha = DveOpSpec(
        name=name, opcode=row, uops=dve_lower(spec, ver=ver), rd1_en=False
    ).sha(ver)
    op = dve_ops.DveOp(name, spec, subdim=False, uops_sha={ver: sha})
    dve_ops.OPS.append(op)
    dve_ops.CUSTOM_DVE_SPECS[name] = spec
    dve_ops._SUB_OPCODE_FOR_NAME[name] = row
    _EXP32_OP = op
    return op


def build_nc():
    exp32 = _register_exp32()
    nc = bacc.Bacc(trn_type="TRN2", target_bir_lowering=False)

    xT_d = nc.dram_tensor("xT", [C, T], MMDT, kind="ExternalInput")
    wqkvT_d = nc.dram_tensor("wqkvT", [C, 3 * P], MMDT, kind="ExternalInput")
    bqkv_d = nc.dram_tensor("bqkv", [P, 3], F32, kind="ExternalInput")
    wprojT_d = nc.dram_tensor("wprojT", [C, C], MMDT, kind="ExternalInput")
    out_d = nc.dram_tensor("out", [2 * P, C], F32, kind="ExternalOutput")

    from contextlib import ExitStack

    with tile.TileContext(nc) as tc, ExitStack() as stack:
        consts = stack.enter_context(tc.tile_pool(name="consts", bufs=1))
        wpool = stack.enter_context(tc.tile_pool(name="wpool", bufs=1))
        main = stack.enter_context(tc.tile_pool(name="main", bufs=1))
        ps_mm = stack.enter_context(tc.tile_pool(name="ps_mm", bufs=2, space="PSUM"))
        ps_att = stack.enter_context(tc.tile_pool(name="ps_att", bufs=2, space="PSUM"))
        ps_y = stack.enter_context(tc.tile_pool(name="ps_y", bufs=2, space="PSUM"))

        # ---- input DMA ----
        wqkvT_s = wpool.tile([P, 8, 3 * P], MMDT)
        wprojT_s = wpool.tile([P, 8, C], MMDT)
        # x quarter 0 split in half-tiles so the very first projection
        # chain waits on only 0.5 MB; quarters as separate tiles
        xq0h = [wpool.tile([P, 8, 256], MMDT, name=f"xq0{h}") for h in range(2)]
        xq_s = [None] + [wpool.tile([P, 8, 512], MMDT, name=f"xq{q}")
                         for q in range(1, 4)]
        xT_r = xT_d.ap().rearrange("(fo p) t -> p fo t", p=P)

        # All bulk transfers serialize at the SDMA engines, so the sync
        # queue carries them in exact need-order; only the tiny bias rides
        # the scalar queue.
        wq_r = wqkvT_d.ap().rearrange("(fo p) n -> p fo n", p=P)
        bqkv_s = consts.tile([P, 3], F32)
        nc.sync.dma_start(out=wqkvT_s[:, :, 0:P], in_=wq_r[:, :, 0:P])
        nc.sync.dma_start(out=xq0h[0], in_=xT_r[:, :, 0:256])
        nc.sync.dma_start(out=bqkv_s, in_=bqkv_d.ap())
        nc.sync.dma_start(out=wqkvT_s[:, :, P:3 * P], in_=wq_r[:, :, P:3 * P])
        nc.sync.dma_start(out=xq0h[1], in_=xT_r[:, :, 256:512])
        for qtr in range(1, 4):
            ts = slice(512 * qtr, 512 * (qtr + 1))
            nc.sync.dma_start(out=xq_s[qtr], in_=xT_r[:, :, ts])
        nc.sync.dma_start(
            out=wprojT_s, in_=wprojT_d.ap().rearrange("(fo p) n -> p fo n", p=P)
        )
        # ---- constants / setup (overlaps the DMA head) ----
        with nc.named_scope("setup"):
            identity = consts.tile([P, P], F32)
            make_identity(nc, identity)
            # shared triangular mask: tri[p, c] = 1.0 if p <= c else 0.0
            tri = consts.tile([P, P], MMDT)
            nc.gpsimd.memset(tri, 1.0)
            nc.gpsimd.affine_select(
                out=tri, in_=tri,
                compare_op=mybir.AluOpType.is_ge,
                fill=0.0, base=0, pattern=[[1, P]], channel_multiplier=-1,
            )
            # preload the Exp table set while DMA streams in
            expwarm = consts.tile([1, 1], F32)
            nc.gpsimd.memset(expwarm, 0.0)
            nc.scalar.activation(expwarm, expwarm, EXPF)
            # PE warmup against the HAM clock gate: ~6.8us of dummy matmuls
            # during the x DMA wait so real matmuls start at 2.4 GHz
            # dummy matmuls stretch PE warmth across the x-DMA wait so the
            # HAM MID window never sees >3.4us of idle before real work
            wps = ps_mm.tile([P, P], F32, name="warm_ps", tag="mm")
            for w in range(30):
                nc.tensor.matmul(
                    wps, lhsT=identity, rhs=identity,
                    start=(w == 0), stop=(w == 29),
                )

        # per-head q/k tiles, zero-padded so the unpacked K=128 q@k
        # contracts only its own head's 64 dims (head A data in partitions
        # 0:64 of qA/kA, head B in 64:128 of qB/kB)
        qA = main.tile([P, T], MMDT, name="qA")
        qB = main.tile([P, T], MMDT, name="qB")
        kA = main.tile([P, T], MMDT, name="kA")
        kB = main.tile([P, T], MMDT, name="kB")
        if not PACK_QK:
            nc.vector.memset(qA[HD:P, :], 0.0)
            nc.vector.memset(qB[0:HD, :], 0.0)
            nc.vector.memset(kA[HD:P, :], 0.0)
            nc.vector.memset(kB[0:HD, :], 0.0)
        q_tiles, k_tiles = (qA, qB), (kA, kB)
        v_t = main.tile([P, T], F32)
        # v_aug: per k-block [t_k, 128]: cols 0:64 = v, col 64 = ones
        # (denominator), rest zero-padded so LDWEIGHTS stays FWL-eligible
        v_augA = main.tile([P, 16, P], MMDT, name="v_augA")
        v_augB = main.tile([P, 16, P], MMDT, name="v_augB")
        v_augs = (v_augA, v_augB)
        nc.gpsimd.memset(v_augA[:, :, HD:P], 0.0)
        nc.gpsimd.memset(v_augB[:, :, HD:P], 0.0)
        nc.gpsimd.memset(v_augA[:, :, HD:HD + 1], 1.0)
        nc.gpsimd.memset(v_augB[:, :, HD:HD + 1], 1.0)
        Y = main.tile([P, 8, 2 * P], MMDT)  # [c'_lo, c'_mid, r_local]
        Y5 = Y.rearrange("p mo (l d two) -> p mo l d two", l=2, d=HD)

        def qkv_group(nt, g):
            """One projection chain + PSUM evacuation (nt=0: two halves)."""
            with nc.named_scope("qkv"):
                if nt == 0:
                    chunks = [(xq0h[0][:, :, :], 0, 256), (xq0h[1][:, :, :], 256, 256)]
                else:
                    chunks = [(xq_s[nt][:, :, :], 0, 512)]
                for xsrc, c0, w in chunks:
                    ts = slice(512 * nt + c0, 512 * nt + c0 + w)
                    ps = ps_mm.tile([P, w], F32, name="ps_mm", tag="mm")
                    for f in range(8):
                        nc.tensor.matmul(
                            ps,
                            lhsT=wqkvT_s[:, f, P * g:P * (g + 1)],
                            rhs=xsrc[:, f, :],
                            start=(f == 0),
                            stop=(f == 7),
                        )
                    if g < 2:
                        dA, dB = (qA, qB) if g == 0 else (kA, kB)
                        nc.scalar.activation(
                            dA[0:HD, ts], ps[0:HD, :], IDENT,
                            bias=bqkv_s[0:HD, g:g + 1])
                        nc.vector.tensor_scalar_add(
                            dB[HD:P, ts], ps[HD:P, :], bqkv_s[HD:P, g:g + 1])
                    else:
                        nc.vector.tensor_scalar_add(
                            v_t[:, ts], ps, bqkv_s[:, 2:3])

        def qk_matmul(l, bk, off, bq, att_ps):
            """att_T[t_k, t_q] partial tile for head l."""
            h = bk % 2
            if PACK_QK:
                half = slice(l * HD, (l + 1) * HD)
            else:
                half = slice(0, P)
            nc.tensor.matmul(
                att_ps[l][:, 512 * h + off:512 * (h + 1)],
                lhsT=k_tiles[l][half, P * bk:P * (bk + 1)],
                rhs=q_tiles[l][half, 512 * bq + off:512 * (bq + 1)],
                start=True,
                stop=True,
            )

        def attn_round(bq, ytp, smallp, expp, inject, post_A=None):
            """Attention round bq; `inject` is a list of closures (next
            round's qkv groups) fed into the pair loop as PE filler;
            `post_A` is emitted between head A's and head B's y-paths."""
            inject = list(inject)
            with nc.named_scope("attn"):
                # v_t -> v_aug transposes for the 4 new t_k blocks
                for b in range(4 * bq, 4 * bq + 4):
                    tp = ps_mm.tile([P, 512], F32, name="ps_tr", tag="mm")
                    nc.tensor.transpose(
                        tp[:, 0:P], v_t[:, P * b:P * (b + 1)], identity
                    )
                    nc.scalar.activation(v_augA[:, b, 0:HD], tp[:, 0:HD], COPYF)
                    nc.vector.tensor_copy(v_augB[:, b, 0:HD], tp[:, HD:2 * HD])
                nbk = 4 * bq + 4
                y_ps = [
                    ps_y.tile([P, 512], F32, name=f"y_ps{l}", tag="y")
                    for l in range(HPC)
                ]

                def av_pair(pair, offs, es_l):
                    with nc.named_scope("attn"):
                        for l in range(HPC):
                            for h in range(2):
                                bk = 2 * pair + h
                                nc.tensor.matmul(
                                    y_ps[l][:, offs[h]:512],
                                    lhsT=v_augs[l][:, bk, :],
                                    rhs=es_l[l][:, 512 * h + offs[h]:512 * (h + 1)],
                                    start=(bk == 0),
                                    stop=(bk == nbk - 1),
                                )

                pending_av = None
                for pair in range(nbk // 2):
                    att_ps = [
                        ps_att.tile([P, 1024], F32, name=f"att_ps{l}", tag="att")
                        for l in range(HPC)
                    ]
                    offs = []
                    for h in range(2):
                        bk = 2 * pair + h
                        rel = bk - 4 * bq
                        off = 128 * rel if rel >= 0 else 0
                        offs.append(off)
                        for l in range(HPC):
                            qk_matmul(l, bk, off, bq, att_ps)
                    diag = 2 * pair >= 4 * bq
                    es_l = []
                    for l in range(HPC):
                        es = expp.tile([P, 1024], MMDT, name="es", tag="es")
                        # head A -> ScalarE exp; head B -> DVE approx, except
                        # round 0 where ScalarE has slack (scale=32 undoes the
                        # 1/32 pre-scale; per-(round,head) consistency holds)
                        use_sc = (l == 0) or (bq == 0)
                        sc = 1.0 if l == 0 else BDIV
                        if not diag:
                            if use_sc:
                                nc.scalar.activation(es, att_ps[l], EXPF, scale=sc)
                            else:
                                nc.vector._custom_dve(exp32, out=es, in0=att_ps[l])
                        else:
                            for h in range(2):
                                sl = slice(512 * h + offs[h], 512 * (h + 1))
                                if use_sc:
                                    nc.scalar.activation(
                                        es[:, sl], att_ps[l][:, sl], EXPF, scale=sc)
                                else:
                                    nc.vector._custom_dve(
                                        exp32, out=es[:, sl], in0=att_ps[l][:, sl])
                        es_l.append(es)
                    if diag:
                        # triangular mask on both 128-wide diagonal sub-blocks
                        # of the pair in one strided 3-D DVE multiply
                        o0 = offs[0]
                        for l in range(HPC):
                            es2 = es_l[l]
                            s1 = es2.ap[-1][0]
                            sl3 = bass.AP(
                                tensor=es2.tensor,
                                offset=es2.offset + o0 * s1,
                                ap=[es2.ap[0], [640 * s1, 2], [s1, P]],
                            )
                            tri3 = bass.AP(
                                tensor=tri.tensor, offset=tri.offset,
                                ap=[tri.ap[0], [0, 2], tri.ap[-1]],
                            )
                            nc.vector.tensor_mul(sl3, sl3, tri3)
                    # software pipeline: previous pair's av lands here, after
                    # this pair's qk has been emitted (in-order PE filler),
                    # with a next-round qkv group as extra cover
                    if inject:
                        inject.pop(0)()
                    if pending_av is not None:
                        av_pair(*pending_av)
                    pending_av = (pair, offs, es_l)
                if pending_av is not None:
                    av_pair(*pending_av)
                for fn in inject:
                    fn()
                # y_T -> y_nat, normalize, scatter into Y. Both PSUM->SBUF
                # copies first so the ps_y ring frees for the typ transposes.
                yts_l = []
                for l in range(HPC):
                    yts = ytp.tile([HD + 1, 512], F32, name="yts", tag="yts")
                    if l == 0:
                        nc.scalar.activation(yts, y_ps[l][0:HD + 1, :], COPYF)
                    else:
                        nc.vector.tensor_copy(yts, y_ps[l][0:HD + 1, :])
                    yts_l.append(yts)
                for l in range(HPC):
                    if l == 1 and post_A is not None:
                        post_A()
                    for sub in range(4):
                        typ = ps_y.tile([P, 512], F32, name="ps_ty", tag="y")
                        nc.tensor.transpose(
                            typ[:, 0:HD + 1],
                            yts_l[l][:, P * sub:P * (sub + 1)],
                            identity[0:HD + 1, 0:HD + 1],
                        )
                        rc = smallp.tile([P, 1], F32, name="rc", tag="rc")
                        nc.vector.reciprocal(rc, typ[:, HD:HD + 1])
                        tb = 4 * bq + sub
                        phalf, mo = tb // 8, tb % 8
                        if l == 0:
                            nc.scalar.activation(
                                Y5[:, mo, l, :, phalf], typ[:, 0:HD], IDENT,
                                scale=rc,
                            )
                        else:
                            nc.vector.tensor_scalar_mul(
                                Y5[:, mo, l, :, phalf], typ[:, 0:HD], rc
                            )

        with (
            tc.tile_pool(name="expp", bufs=8) as expp,
            tc.tile_pool(name="ytp", bufs=4) as ytp,
            tc.tile_pool(name="smallp", bufs=8) as smallp,
            tc.tile_pool(name="outp", bufs=2) as outp,
        ):
            def proj_mt(mt):
                # output projection rows for one head (row-parallel);
                # bias is added host-side, result DMAs on alternating queues
                with nc.named_scope("proj"):
                    for nt in range(2):
                        ps = ps_mm.tile([P, 512], F32, name="ps_pr", tag="mm")
                        for mo in range(8):
                            nc.tensor.matmul(
                                ps,
                                lhsT=Y[:, mo, P * mt:P * (mt + 1)],
                                rhs=wprojT_s[:, mo, 512 * nt:512 * (nt + 1)],
                                start=(mo == 0),
                                stop=(mo == 7),
                            )
                        os_ = outp.tile([P, 512], F32, name="os", tag="os")
                        if nt == 0:
                            nc.scalar.activation(os_, ps, COPYF)
                        else:
                            nc.vector.tensor_copy(os_, ps)
                        eng = nc.sync if nt == 0 else nc.scalar
                        eng.dma_start(
                            out=out_d.ap()[P * mt:P * (mt + 1), 512 * nt:512 * (nt + 1)],
                            in_=os_,
                        )

            # round 0's projections up front; round r+1's are injected into
            # round r's pair loop as PE filler behind attention stalls.
            # Round 3 emits head A's output projection between the heads'
            # y-paths so the tail shortens.
            for g in range(3):
                qkv_group(0, g)
            for r in range(4):
                if r < 3:
                    inject = [lambda nt=r + 1, g=g: qkv_group(nt, g)
                              for g in range(3)]
                else:
                    inject = []
                attn_round(r, ytp, smallp, expp, inject,
                           post_A=(lambda: proj_mt(0)) if r == 3 else None)
            proj_mt(1)

    nc.compile()
    return nc


_nc_cache = None


def kernel(**inputs):
    global _nc_cache
    x = np.ascontiguousarray(np.asarray(inputs["x"], dtype=np.float32))
    w_attn = np.asarray(inputs["w_attn"], dtype=np.float32)
    b_attn = np.asarray(inputs["b_attn"], dtype=np.float32)
    w_proj = np.asarray(inputs["w_proj"], dtype=np.float32)
    b_proj = np.asarray(inputs["b_proj"], dtype=np.float32)

    scale = 1.0 / math.sqrt(HD)
    xT = _to_mm(x.T)
    wprojT = _to_mm(w_proj.T)

    in_maps = []
    for c in range(NCORES):
        ch0 = P * c
        # head A (rows 0:64): plain 1/sqrt(HD) scale for ScalarE exp;
        # head B (rows 64:128): extra 1/32 so the DVE approx sees a/32
        qsc = np.full((P, 1), scale, dtype=np.float32)
        qsc[HD:P] /= BDIV
        wq = w_attn[ch0:ch0 + P, :] * qsc
        wk = w_attn[C + ch0:C + ch0 + P, :]
        wv = w_attn[2 * C + ch0:2 * C + ch0 + P, :]
        wqkvT = _to_mm(np.concatenate([wq, wk, wv], axis=0).T)
        bqkv = np.ascontiguousarray(
            np.stack(
                [
                    b_attn[ch0:ch0 + P] * qsc[:, 0],
                    b_attn[C + ch0:C + ch0 + P],
                    b_attn[2 * C + ch0:2 * C + ch0 + P],
                ],
                axis=1,
            )
        )
        in_maps.append(
            {
                "xT": xT,
                "wqkvT": wqkvT,
                "bqkv": bqkv,
                "wprojT": wprojT,
            }
        )

    if _nc_cache is None:
        _nc_cache = build_nc()
    nc = _nc_cache

    trace = os.environ.get("BASS_KERNEL_TRACE", "0") == "1"
    res = bass_utils.run_bass_kernel_spmd(
        nc, in_maps, core_ids=list(range(NCORES)), trace=trace
    )
    if trace:
        print(f"HW exec time: {res.exec_time_ns} ns")
        if res.per_core_scope_times:
            for scope, times in sorted(res.per_core_scope_times.items()):
                print(f"  scope {scope}: {times}")
        if res.instructions_and_trace:
            print(f"  trace: {res.instructions_and_trace[1]}")

    out = np.concatenate([r["out"] for r in res.results], axis=0)
    out = out + b_proj[None, :]
    return np.ascontiguousarray(out.astype(np.float32))


if __name__ == "__main__":
    nc = build_nc()
    print("build OK")
